# revision 14
# baseline (speedup 1.0000x reference)
"""Trainium2 Bass kernel for nn_DPLoss (histogram_binning).

Data-parallel over batch: 2 batches per core on 8 cores. Per batch b,
class c (C=4, only c>=1 contribute) the device computes
  D_c = sum_p [t==c] * (x_c - lse)      (= A_c - B_c, CE numerator)
  H_c = sum_p [x_c == max_c' x_c']      (pred histogram, fp16 compare)
with lse = log(sum_c exp(x_c)).  Host combines:
  loss = sum_{b,c>=1} w[b,c] * (-D_c - H_c) / (H*W)
  w = sigmoid(bw); w /= w.mean(axis=0); w /= (1+e)

v3: x is cast to fp16 during the SWDGE load (HBM traffic unchanged,
fp16 keeps argmax ties rare: ~3e-4 total rel err).  All DVE ops run in
bf16/fp16 fast modes (TS 4x, TT 2x); there are NO slow 1x accumulate
ops: every reduction happens on the PE as a ones-vector matmul
accumulating into one packed PSUM tile ([12, 512], row = b*6 + q).

Engine split per [128, SW] stage:
  DMA gpsimd (SWDGE, f32->fp16 cast): 4 class planes; sync: target i32
  ACT: t16 = Copy(t); E_c = exp(x_c) fp16; L = log(S) fp16
  PE : S = sum_c E_c (identity matmuls); 6 ones-matmul reductions/chunk
  DVE: 3 TS masks, 3 TT max-tree, 3 TT eq, 3 TT (x-L), 3 TT mask*(x-L)
"""

import numpy as np

_B, _C, _H, _W = 16, 4, 768, 768
_HW = _H * _W            # 589824
_NCORES = 8
_NB = _B // _NCORES      # 2 batches per core
_P = 128
_FREE = _HW // _P        # 4608
_SW = 1536               # stage width (free-dim columns per tile)
_NS = _FREE // _SW       # 3 stages per batch
_CHUNK = 512             # psum / matmul chunk
_NCH = _SW // _CHUNK
_NQ = 3                  # D1,D2,D3 (H via ACT accum)
_NROW = _NB * _NQ        # rows of the packed psum accumulator

_nc_cache = None


def _patch_act_tables():
    """Force a single activation table set (has Exp, Ln, Copy) so the
    compiler doesn't thrash table loads between Exp and Ln sets."""
    import concourse.bacc as bacc_mod
    import concourse.hw_specs as hw_specs

    if getattr(bacc_mod, "_act_tables_patched", False):
        return
    orig = hw_specs.get_activation_tables

    def patched(module_arch):
        t = orig(module_arch)
        keep = "natural_log_exp_and_others"
        return {k: (v if k == keep else set()) for k, v in t.items()}

    bacc_mod.get_activation_tables = patched
    bacc_mod._act_tables_patched = True


def _build():
    import concourse.tile as tile
    from concourse import bacc, mybir

    _patch_act_tables()

    f32 = mybir.dt.float32
    f16 = mybir.dt.float16
    i32 = mybir.dt.int32
    AF = mybir.ActivationFunctionType
    OP = mybir.AluOpType

    nc = bacc.Bacc(
        "TRN2",
        target_bir_lowering=False,
        debug=False,
        enable_asserts=False,
        num_devices=_NCORES,
    )
    x = nc.dram_tensor("x", [_NB, _C, _P, _FREE], f32, kind="ExternalInput").ap()
    t = nc.dram_tensor("t", [_NB, _P, _FREE], i32, kind="ExternalInput").ap()
    cst = nc.dram_tensor("c", [_P, 256 + _NROW * _NROW], f16,
                         kind="ExternalInput").ap()
    out = nc.dram_tensor("o", [_NROW, _CHUNK], f32, kind="ExternalOutput").ap()
    outh = nc.dram_tensor("oh", [_P, 3 * _NB * _NS * _NCH], f32,
                          kind="ExternalOutput").ap()

    with tile.TileContext(nc) as tc:
        with (
            tc.tile_pool(name="const", bufs=1) as constp,
            tc.tile_pool(name="xin", bufs=3) as xin,
            tc.tile_pool(name="tin", bufs=3) as tin,
            tc.tile_pool(name="ework", bufs=3) as ework,
            tc.tile_pool(name="work", bufs=2) as work,
            tc.tile_pool(name="prodp", bufs=2) as prodp,
            tc.tile_pool(name="outp", bufs=1) as outp,
            tc.tile_pool(name="ps", bufs=3, space="PSUM") as ps,
            tc.tile_pool(name="psacc", bufs=1, space="PSUM") as psacc,
        ):
            # consts (one DMA): identity [128,128] then selq blocks — selq
            # block q ([128, NROW]) has ones in column q only: a ones-reduce
            # matmul with it lands the column-sums in psum row q and adds
            # zeros to every other row.
            cstt = constp.tile([_P, 256 + _NROW * _NROW], f16)
            nc.sync.dma_start(cstt[:], cst)
            ident = cstt[:, 0:128]
            nident = cstt[:, 128:256]
            selq = cstt[:, 256:]
            accps = psacc.tile([_NROW, _CHUNK], f32)
            acch = outp.tile([_P, 3 * _NB * _NS * _NCH], f32)

            for b in range(_NB):
                for s in range(_NS):
                    sl = slice(s * _SW, (s + 1) * _SW)
                    first = s == 0
                    last = s == _NS - 1

                    xt = []
                    for c in range(_C):
                        xc = xin.tile([_P, _SW], f16, tag=f"x{c}")
                        nc.gpsimd.dma_start(xc[:], x[b, c, :, sl])
                        xt.append(xc)
                    tb = tin.tile([_P, _SW], f16, tag="tb")
                    nc.gpsimd.dma_start(tb[:], t[b, :, sl])

                    # --- lse = log(sum_c exp(x_c)) ---
                    et = []
                    for c in range(_C):
                        ec = ework.tile([_P, _SW], f16, tag=f"e{c}")
                        nc.scalar.activation(ec[:], xt[c][:], AF.Exp)
                        et.append(ec)
                    L = work.tile([_P, _SW], f16, tag="L")
                    for ch in range(_NCH):
                        chs = slice(ch * _CHUNK, (ch + 1) * _CHUNK)
                        S = ps.tile([_P, _CHUNK], f32, tag="S")
                        for c in range(_C):
                            nc.tensor.matmul(
                                S[:], ident, et[c][:, chs],
                                start=(c == 0), stop=(c == _C - 1),
                            )
                        nc.scalar.activation(L[:, chs], S[:], AF.Ln)

                    # --- masks (TS 4x) ---
                    mk = []
                    for c in (1, 2, 3):
                        mc = work.tile([_P, _SW], f16, tag=f"m{c}")
                        nc.vector.tensor_scalar(
                            mc[:], tb[:], float(c), None, op0=OP.is_equal)
                        mk.append(mc)

                    # --- max tree (TT 2x) ---
                    m01 = work.tile([_P, _SW], f16, tag="m01")
                    nc.vector.tensor_tensor(m01[:], xt[0][:], xt[1][:], op=OP.max)
                    m23 = work.tile([_P, _SW], f16, tag="m23")
                    nc.vector.tensor_tensor(m23[:], xt[2][:], xt[3][:], op=OP.max)
                    M = work.tile([_P, _SW], f16, tag="M")
                    nc.vector.tensor_tensor(M[:], m01[:], m23[:], op=OP.max)

                    # --- per-class product tiles (TT 2x) ---
                    tiles_q = []
                    for i, c in enumerate((1, 2, 3)):
                        dc = prodp.tile([_P, _SW], f16, tag=f"d{c}")
                        nc.vector.tensor_tensor(dc[:], xt[c][:], L[:], op=OP.subtract)
                        pc = prodp.tile([_P, _SW], f16, tag=f"p{c}")
                        nc.vector.tensor_tensor(pc[:], mk[i][:], dc[:], op=OP.mult)
                        tiles_q.append(pc)  # q = 0,1,2 -> D_c
                    # hist via PE (x_c - M in psum, exact) + ACT exp(K*D)
                    for i2, c in enumerate((1, 2, 3)):
                        for ch in range(_NCH):
                            chs = slice(ch * _CHUNK, (ch + 1) * _CHUNK)
                            Dp = ps.tile([_P, _CHUNK], f32, tag="D")
                            nc.tensor.matmul(
                                Dp[:], ident, xt[c][:, chs],
                                start=True, stop=False)
                            nc.tensor.matmul(
                                Dp[:], nident, M[:, chs],
                                start=False, stop=True)
                            hs = prodp.tile([_P, _CHUNK], f16, tag="hs")
                            col = ((b * _NS + s) * _NCH + ch) * 3 + i2
                            nc.scalar.activation(
                                hs[:], Dp[:], AF.Exp, scale=1e9,
                                accum_out=acch[:, col:col + 1])

                    # --- PE reductions into packed psum rows ---
                    for q, tq in enumerate(tiles_q):
                        row = b * _NQ + q
                        sel = selq[:, row * _NROW: (row + 1) * _NROW]
                        for ch in range(_NCH):
                            chs = slice(ch * _CHUNK, (ch + 1) * _CHUNK)
                            glob_first = b == 0 and first and q == 0 and ch == 0
                            glob_last = (b == _NB - 1 and last
                                         and q == _NQ - 1 and ch == _NCH - 1)
                            nc.tensor.matmul(
                                accps[:, :], sel, tq[:, chs],
                                start=glob_first, stop=glob_last,
                                skip_group_check=True,
                            )

            res = outp.tile([_NROW, _CHUNK], f32)
            nc.vector.tensor_copy(res[:], accps[:])
            nc.sync.dma_start(out[:, :], res[:])
            nc.sync.dma_start(outh[:, :], acch[:])
    nc.compile()
    return nc


def _get_nc():
    global _nc_cache
    if _nc_cache is None:
        _nc_cache = _build()
    return _nc_cache


def _make_consts():
    import ml_dtypes

    cst = np.zeros((_P, 256 + _NROW * _NROW), np.float32)
    cst[:, :128] = np.eye(128, dtype=np.float32)
    cst[:, 128:256] = -np.eye(128, dtype=np.float32)
    for q in range(_NROW):
        cst[:, 256 + q * _NROW + q] = 1.0
    return cst.astype(ml_dtypes.float16 if hasattr(ml_dtypes, "float16") else np.float16)


def _make_in_maps(net_output, target):
    net_output = np.ascontiguousarray(net_output, dtype=np.float32)
    target = np.ascontiguousarray(target, dtype=np.int32)
    cst = np.ascontiguousarray(_make_consts())
    in_maps = []
    for k in range(_NCORES):
        xs = net_output[_NB * k: _NB * (k + 1)].reshape(_NB, _C, _P, _FREE)
        ts = target[_NB * k: _NB * (k + 1), 0].reshape(_NB, _P, _FREE)
        in_maps.append({"x": np.ascontiguousarray(xs), "t": np.ascontiguousarray(ts),
                        "c": cst})
    return in_maps


def _combine(results, bare_weight):
    # results: list of dicts with 'o' [NROW, CHUNK] per core
    D = np.zeros((_B, _C), np.float64)
    Hc = np.zeros((_B, _C), np.float64)
    for k, r in enumerate(results):
        o = r["o"].astype(np.float64).sum(axis=1).reshape(_NB, _NQ)
        oh = r["oh"].astype(np.float64).sum(axis=0).reshape(_NB, _NS * _NCH, 3)
        ohb = oh.sum(axis=1)
        for bb in range(_NB):
            gb = _NB * k + bb
            D[gb, 1:4] = o[bb, 0:3]
            Hc[gb, 1:4] = ohb[bb]

    bw = bare_weight.astype(np.float64)
    sig = 1.0 / (1.0 + np.exp(-bw))
    w = sig / sig.mean(axis=0, keepdims=True)
    w = w / (1.0 + np.e)  # fixed_w for classes >= 1
    loss = (w[:, 1:] * (-D[:, 1:] - Hc[:, 1:])).sum() / _HW
    return np.float32(loss)


def _run(net_output, target, bare_weight, **spmd_kwargs):
    from concourse.bass_utils import run_bass_kernel_spmd

    nc = _get_nc()
    in_maps = _make_in_maps(net_output, target)
    res = run_bass_kernel_spmd(nc, in_maps, core_ids=list(range(_NCORES)), **spmd_kwargs)
    return _combine(res.results, np.asarray(bare_weight)), res


def kernel(net_output, target, bare_weight):
    loss, _ = _run(np.asarray(net_output), np.asarray(target), np.asarray(bare_weight))
    return loss


# revision 15
# speedup vs baseline: 1.1611x; 1.1611x over previous
"""Trainium2 Bass kernel for nn_DPLoss (histogram_binning).

Data-parallel over batch: 2 batches per core on 8 cores. Per batch b,
class c (C=4, only c>=1 contribute) the device computes
  D_c = sum_p [t==c] * (x_c - lse)      (= A_c - B_c, CE numerator)
  H_c = sum_p [x_c == max_c' x_c']      (pred histogram, fp16 compare)
with lse = log(sum_c exp(x_c)).  Host combines:
  loss = sum_{b,c>=1} w[b,c] * (-D_c - H_c) / (H*W)
  w = sigmoid(bw); w /= w.mean(axis=0); w /= (1+e)

v3: x is cast to fp16 during the SWDGE load (HBM traffic unchanged,
fp16 keeps argmax ties rare: ~3e-4 total rel err).  All DVE ops run in
bf16/fp16 fast modes (TS 4x, TT 2x); there are NO slow 1x accumulate
ops: every reduction happens on the PE as a ones-vector matmul
accumulating into one packed PSUM tile ([12, 512], row = b*6 + q).

Engine split per [128, SW] stage:
  DMA gpsimd (SWDGE, f32->fp16 cast): 4 class planes; sync: target i32
  ACT: t16 = Copy(t); E_c = exp(x_c) fp16; L = log(S) fp16
  PE : S = sum_c E_c (identity matmuls); 6 ones-matmul reductions/chunk
  DVE: 3 TS masks, 3 TT max-tree, 3 TT eq, 3 TT (x-L), 3 TT mask*(x-L)
"""

import numpy as np

_B, _C, _H, _W = 16, 4, 768, 768
_HW = _H * _W            # 589824
_NCORES = 8
_NB = _B // _NCORES      # 2 batches per core
_P = 128
_FREE = _HW // _P        # 4608
_SW = 1536               # stage width (free-dim columns per tile)
_NS = _FREE // _SW       # 3 stages per batch
_CHUNK = 512             # psum / matmul chunk
_NCH = _SW // _CHUNK
_NQ = 6                  # D1,D2,D3,H1,H2,H3
_NROW = _NB * _NQ        # rows of the packed psum accumulator

_nc_cache = None


def _patch_act_tables():
    """Force a single activation table set (has Exp, Ln, Copy) so the
    compiler doesn't thrash table loads between Exp and Ln sets."""
    import concourse.bacc as bacc_mod
    import concourse.hw_specs as hw_specs

    if getattr(bacc_mod, "_act_tables_patched", False):
        return
    orig = hw_specs.get_activation_tables

    def patched(module_arch):
        t = orig(module_arch)
        keep = "natural_log_exp_and_others"
        return {k: (v if k == keep else set()) for k, v in t.items()}

    bacc_mod.get_activation_tables = patched
    bacc_mod._act_tables_patched = True


def _build():
    import concourse.tile as tile
    from concourse import bacc, mybir

    _patch_act_tables()

    f32 = mybir.dt.float32
    f16 = mybir.dt.float16
    i32 = mybir.dt.int32
    AF = mybir.ActivationFunctionType
    OP = mybir.AluOpType

    nc = bacc.Bacc(
        "TRN2",
        target_bir_lowering=False,
        debug=False,
        enable_asserts=False,
        num_devices=_NCORES,
    )
    x = nc.dram_tensor("x", [_NB, _C, _P, _FREE], f32, kind="ExternalInput").ap()
    t = nc.dram_tensor("t", [_NB, _P, _FREE], i32, kind="ExternalInput").ap()
    cst = nc.dram_tensor("c", [_P, 128 + _NROW * _NROW], f16,
                         kind="ExternalInput").ap()
    out = nc.dram_tensor("o", [_NROW, _CHUNK], f32, kind="ExternalOutput").ap()

    with tile.TileContext(nc) as tc:
        with (
            tc.tile_pool(name="const", bufs=1) as constp,
            tc.tile_pool(name="xin", bufs=3) as xin,
            tc.tile_pool(name="tin", bufs=3) as tin,
            tc.tile_pool(name="ework", bufs=3) as ework,
            tc.tile_pool(name="work", bufs=2) as work,
            tc.tile_pool(name="prodp", bufs=2) as prodp,
            tc.tile_pool(name="outp", bufs=1) as outp,
            tc.tile_pool(name="ps", bufs=3, space="PSUM") as ps,
            tc.tile_pool(name="psacc", bufs=1, space="PSUM") as psacc,
        ):
            # consts (one DMA): identity [128,128] then selq blocks — selq
            # block q ([128, NROW]) has ones in column q only: a ones-reduce
            # matmul with it lands the column-sums in psum row q and adds
            # zeros to every other row.
            cstt = constp.tile([_P, 128 + _NROW * _NROW], f16)
            nc.sync.dma_start(cstt[:], cst)
            ident = cstt[:, 0:128]
            selq = cstt[:, 128:]
            accps = psacc.tile([_NROW, _CHUNK], f32)

            for b in range(_NB):
                for s in range(_NS):
                    sl = slice(s * _SW, (s + 1) * _SW)
                    first = s == 0
                    last = s == _NS - 1

                    tb = tin.tile([_P, _SW], f16, tag="tb")
                    nc.gpsimd.dma_start(tb[:], t[b, :, sl])
                    xt = []
                    for c in range(_C):
                        xc = xin.tile([_P, _SW], f16, tag=f"x{c}")
                        nc.gpsimd.dma_start(xc[:], x[b, c, :, sl])
                        xt.append(xc)

                    # --- masks (TS 4x) ---
                    mk = []
                    for c in (1, 2, 3):
                        mc = work.tile([_P, _SW], f16, tag=f"m{c}")
                        nc.vector.tensor_scalar(
                            mc[:], tb[:], float(c), None, op0=OP.is_equal)
                        mk.append(mc)

                    # --- max tree (TT 2x) ---
                    m01 = work.tile([_P, _SW], f16, tag="m01")
                    nc.vector.tensor_tensor(m01[:], xt[0][:], xt[1][:], op=OP.max)
                    m23 = work.tile([_P, _SW], f16, tag="m23")
                    nc.vector.tensor_tensor(m23[:], xt[2][:], xt[3][:], op=OP.max)
                    M = work.tile([_P, _SW], f16, tag="M")
                    nc.vector.tensor_tensor(M[:], m01[:], m23[:], op=OP.max)

                    # --- lse = log(sum_c exp(x_c)) ---
                    et = []
                    for c in range(_C):
                        ec = ework.tile([_P, _SW], f16, tag=f"e{c}")
                        nc.scalar.activation(ec[:], xt[c][:], AF.Exp)
                        et.append(ec)
                    L = work.tile([_P, _SW], f16, tag="L")
                    for ch in range(_NCH):
                        chs = slice(ch * _CHUNK, (ch + 1) * _CHUNK)
                        S = ps.tile([_P, _CHUNK], f32, tag="S")
                        for c in range(_C):
                            nc.tensor.matmul(
                                S[:], ident, et[c][:, chs],
                                start=(c == 0), stop=(c == _C - 1),
                            )
                        nc.scalar.activation(L[:, chs], S[:], AF.Ln)

                    # --- per-class product tiles (TT 2x) ---
                    tiles_q = []
                    eqs = []
                    for c in (1, 2, 3):
                        ec2 = prodp.tile([_P, _SW], f16, tag=f"q{c}")
                        nc.vector.tensor_tensor(ec2[:], xt[c][:], M[:], op=OP.is_equal)
                        eqs.append(ec2)
                    for i, c in enumerate((1, 2, 3)):
                        dc = prodp.tile([_P, _SW], f16, tag=f"d{c}")
                        nc.vector.tensor_tensor(dc[:], xt[c][:], L[:], op=OP.subtract)
                        pc = prodp.tile([_P, _SW], f16, tag=f"p{c}")
                        nc.vector.tensor_tensor(pc[:], mk[i][:], dc[:], op=OP.mult)
                        tiles_q.append(pc)  # q = 0,1,2 -> D_c
                    tiles_q.extend(eqs)     # q = 3,4,5 -> H_c

                    # --- PE reductions into packed psum rows ---
                    for q, tq in enumerate(tiles_q):
                        row = b * _NQ + q
                        sel = selq[:, row * _NROW: (row + 1) * _NROW]
                        for ch in range(_NCH):
                            chs = slice(ch * _CHUNK, (ch + 1) * _CHUNK)
                            glob_first = b == 0 and first and q == 0 and ch == 0
                            glob_last = (b == _NB - 1 and last
                                         and q == _NQ - 1 and ch == _NCH - 1)
                            nc.tensor.matmul(
                                accps[:, :], sel, tq[:, chs],
                                start=glob_first, stop=glob_last,
                                skip_group_check=True,
                            )

            res = outp.tile([_NROW, _CHUNK], f32)
            nc.vector.tensor_copy(res[:], accps[:])
            nc.sync.dma_start(out[:, :], res[:])
    nc.compile()
    return nc


def _get_nc():
    global _nc_cache
    if _nc_cache is None:
        _nc_cache = _build()
    return _nc_cache


def _make_consts():
    import ml_dtypes

    cst = np.zeros((_P, 128 + _NROW * _NROW), np.float32)
    cst[:, :128] = np.eye(128, dtype=np.float32)
    for q in range(_NROW):
        cst[:, 128 + q * _NROW + q] = 1.0
    return cst.astype(ml_dtypes.float16 if hasattr(ml_dtypes, "float16") else np.float16)


def _make_in_maps(net_output, target):
    net_output = np.ascontiguousarray(net_output, dtype=np.float32)
    target = np.ascontiguousarray(target, dtype=np.int32)
    cst = np.ascontiguousarray(_make_consts())
    in_maps = []
    for k in range(_NCORES):
        xs = net_output[_NB * k: _NB * (k + 1)].reshape(_NB, _C, _P, _FREE)
        ts = target[_NB * k: _NB * (k + 1), 0].reshape(_NB, _P, _FREE)
        in_maps.append({"x": np.ascontiguousarray(xs), "t": np.ascontiguousarray(ts),
                        "c": cst})
    return in_maps


def _combine(results, bare_weight):
    # results: list of dicts with 'o' [NROW, CHUNK] per core
    D = np.zeros((_B, _C), np.float64)
    Hc = np.zeros((_B, _C), np.float64)
    for k, r in enumerate(results):
        o = r["o"].astype(np.float64).sum(axis=1).reshape(_NB, _NQ)
        for bb in range(_NB):
            gb = _NB * k + bb
            D[gb, 1:4] = o[bb, 0:3]
            Hc[gb, 1:4] = o[bb, 3:6]

    bw = bare_weight.astype(np.float64)
    sig = 1.0 / (1.0 + np.exp(-bw))
    w = sig / sig.mean(axis=0, keepdims=True)
    w = w / (1.0 + np.e)  # fixed_w for classes >= 1
    loss = (w[:, 1:] * (-D[:, 1:] - Hc[:, 1:])).sum() / _HW
    return np.float32(loss)


def _run(net_output, target, bare_weight, **spmd_kwargs):
    from concourse.bass_utils import run_bass_kernel_spmd

    nc = _get_nc()
    in_maps = _make_in_maps(net_output, target)
    res = run_bass_kernel_spmd(nc, in_maps, core_ids=list(range(_NCORES)), **spmd_kwargs)
    return _combine(res.results, np.asarray(bare_weight)), res


def kernel(net_output, target, bare_weight):
    loss, _ = _run(np.asarray(net_output), np.asarray(target), np.asarray(bare_weight))
    return loss


# revision 16
# speedup vs baseline: 1.2189x; 1.0498x over previous
"""Trainium2 Bass kernel for nn_DPLoss (histogram_binning).

Data-parallel over batch: 2 batches per core on 8 cores. Per batch b,
class c (C=4, only c>=1 contribute) the device computes
  D_c = sum_p [t==c] * (x_c - lse)      (= A_c - B_c, CE numerator)
  H_c = sum_p [x_c == max_c' x_c']      (pred histogram, fp16 compare)
with lse = log(sum_c exp(x_c)).  Host combines:
  loss = sum_{b,c>=1} w[b,c] * (-D_c - H_c) / (H*W)
  w = sigmoid(bw); w /= w.mean(axis=0); w /= (1+e)

v3: x is cast to fp16 during the SWDGE load (HBM traffic unchanged,
fp16 keeps argmax ties rare: ~3e-4 total rel err).  All DVE ops run in
bf16/fp16 fast modes (TS 4x, TT 2x); there are NO slow 1x accumulate
ops: every reduction happens on the PE as a ones-vector matmul
accumulating into one packed PSUM tile ([12, 512], row = b*6 + q).

Engine split per [128, SW] stage:
  DMA gpsimd (SWDGE, f32->fp16 cast): 4 class planes; sync: target i32
  ACT: t16 = Copy(t); E_c = exp(x_c) fp16; L = log(S) fp16
  PE : S = sum_c E_c (identity matmuls); 6 ones-matmul reductions/chunk
  DVE: 3 TS masks, 3 TT max-tree, 3 TT eq, 3 TT (x-L), 3 TT mask*(x-L)
"""

import numpy as np

_B, _C, _H, _W = 16, 4, 768, 768
_HW = _H * _W            # 589824
_NCORES = 8
_NB = _B // _NCORES      # 2 batches per core
_P = 128
_FREE = _HW // _P        # 4608
_SW = 1536               # stage width (free-dim columns per tile)
_NS = _FREE // _SW       # 3 stages per batch
_CHUNK = 512             # psum / matmul chunk
_NCH = _SW // _CHUNK
_NQ = 6                  # D1,D2,D3,H1,H2,H3
_NROW = _NB * _NQ        # rows of the packed psum accumulator

_nc_cache = None


def _patch_act_tables():
    """Force a single activation table set (has Exp, Ln, Copy) so the
    compiler doesn't thrash table loads between Exp and Ln sets."""
    import concourse.bacc as bacc_mod
    import concourse.hw_specs as hw_specs

    if getattr(bacc_mod, "_act_tables_patched", False):
        return
    orig = hw_specs.get_activation_tables

    def patched(module_arch):
        t = orig(module_arch)
        keep = "natural_log_exp_and_others"
        return {k: (v if k == keep else set()) for k, v in t.items()}

    bacc_mod.get_activation_tables = patched
    bacc_mod._act_tables_patched = True


def _build():
    import concourse.tile as tile
    from concourse import bacc, mybir

    _patch_act_tables()

    f32 = mybir.dt.float32
    f16 = mybir.dt.float16
    i32 = mybir.dt.int32
    AF = mybir.ActivationFunctionType
    OP = mybir.AluOpType

    nc = bacc.Bacc(
        "TRN2",
        target_bir_lowering=False,
        debug=False,
        enable_asserts=False,
        num_devices=_NCORES,
    )
    x = nc.dram_tensor("x", [_NB, _C, _P, _FREE], f32, kind="ExternalInput").ap()
    t = nc.dram_tensor("t", [_NB, _P, _FREE], i32, kind="ExternalInput").ap()
    cst = nc.dram_tensor("c", [_P, 128 + _NROW * _NROW], f16,
                         kind="ExternalInput").ap()
    out = nc.dram_tensor("o", [_NROW, _CHUNK], f32, kind="ExternalOutput").ap()

    with tile.TileContext(nc) as tc:
        with (
            tc.tile_pool(name="const", bufs=1) as constp,
            tc.tile_pool(name="xin", bufs=3) as xin,
            tc.tile_pool(name="tin", bufs=3) as tin,
            tc.tile_pool(name="ework", bufs=3) as ework,
            tc.tile_pool(name="work", bufs=2) as work,
            tc.tile_pool(name="prodp", bufs=2) as prodp,
            tc.tile_pool(name="outp", bufs=1) as outp,
            tc.tile_pool(name="ps", bufs=3, space="PSUM") as ps,
            tc.tile_pool(name="psacc", bufs=1, space="PSUM") as psacc,
        ):
            # consts (one DMA): identity [128,128] then selq blocks — selq
            # block q ([128, NROW]) has ones in column q only: a ones-reduce
            # matmul with it lands the column-sums in psum row q and adds
            # zeros to every other row.
            cstt = constp.tile([_P, 128 + _NROW * _NROW], f16)
            nc.sync.dma_start(cstt[:], cst)
            ident = cstt[:, 0:128]
            selq = cstt[:, 128:]
            accps = psacc.tile([_NROW, _CHUNK], f32)

            for b in range(_NB):
                for s in range(_NS):
                    sl = slice(s * _SW, (s + 1) * _SW)
                    first = s == 0
                    last = s == _NS - 1

                    xt = []
                    for c in range(_C):
                        xc = xin.tile([_P, _SW], f16, tag=f"x{c}")
                        nc.gpsimd.dma_start(xc[:], x[b, c, :, sl])
                        xt.append(xc)
                    tb = tin.tile([_P, _SW], f16, tag="tb")
                    nc.gpsimd.dma_start(tb[:], t[b, :, sl])

                    # --- lse = log(sum_c exp(x_c)) ---
                    et = []
                    for c in range(_C):
                        ec = ework.tile([_P, _SW], f16, tag=f"e{c}")
                        nc.scalar.activation(ec[:], xt[c][:], AF.Exp)
                        et.append(ec)
                    L = work.tile([_P, _SW], f16, tag="L")
                    for ch in range(_NCH):
                        chs = slice(ch * _CHUNK, (ch + 1) * _CHUNK)
                        S = ps.tile([_P, _CHUNK], f32, tag="S")
                        for c in range(_C):
                            nc.tensor.matmul(
                                S[:], ident, et[c][:, chs],
                                start=(c == 0), stop=(c == _C - 1),
                            )
                        nc.scalar.activation(L[:, chs], S[:], AF.Ln)

                    # --- masks (TS 4x) ---
                    mk = []
                    for c in (1, 2, 3):
                        mc = work.tile([_P, _SW], f16, tag=f"m{c}")
                        nc.vector.tensor_scalar(
                            mc[:], tb[:], float(c), None, op0=OP.is_equal)
                        mk.append(mc)

                    # --- max tree (TT 2x) ---
                    m01 = work.tile([_P, _SW], f16, tag="m01")
                    nc.vector.tensor_tensor(m01[:], xt[0][:], xt[1][:], op=OP.max)
                    m23 = work.tile([_P, _SW], f16, tag="m23")
                    nc.vector.tensor_tensor(m23[:], xt[2][:], xt[3][:], op=OP.max)
                    M = work.tile([_P, _SW], f16, tag="M")
                    nc.vector.tensor_tensor(M[:], m01[:], m23[:], op=OP.max)

                    # --- per-class product tiles (TT 2x) ---
                    tiles_q = []
                    for i, c in enumerate((1, 2, 3)):
                        dc = prodp.tile([_P, _SW], f16, tag=f"d{c}")
                        nc.vector.tensor_tensor(dc[:], xt[c][:], L[:], op=OP.subtract)
                        pc = prodp.tile([_P, _SW], f16, tag=f"p{c}")
                        nc.vector.tensor_tensor(pc[:], mk[i][:], dc[:], op=OP.mult)
                        tiles_q.append(pc)  # q = 0,1,2 -> D_c
                    for c in (1, 2, 3):
                        ec2 = prodp.tile([_P, _SW], f16, tag=f"q{c}")
                        nc.vector.tensor_tensor(ec2[:], xt[c][:], M[:], op=OP.is_equal)
                        tiles_q.append(ec2)  # q = 3,4,5 -> H_c

                    # --- PE reductions into packed psum rows ---
                    for q, tq in enumerate(tiles_q):
                        row = b * _NQ + q
                        sel = selq[:, row * _NROW: (row + 1) * _NROW]
                        for ch in range(_NCH):
                            chs = slice(ch * _CHUNK, (ch + 1) * _CHUNK)
                            glob_first = b == 0 and first and q == 0 and ch == 0
                            glob_last = (b == _NB - 1 and last
                                         and q == _NQ - 1 and ch == _NCH - 1)
                            nc.tensor.matmul(
                                accps[:, :], sel, tq[:, chs],
                                start=glob_first, stop=glob_last,
                                skip_group_check=True,
                            )

            res = outp.tile([_NROW, _CHUNK], f32)
            nc.vector.tensor_copy(res[:], accps[:])
            nc.sync.dma_start(out[:, :], res[:])
    nc.compile()
    return nc


def _get_nc():
    global _nc_cache
    if _nc_cache is None:
        _nc_cache = _build()
    return _nc_cache


def _make_consts():
    import ml_dtypes

    cst = np.zeros((_P, 128 + _NROW * _NROW), np.float32)
    cst[:, :128] = np.eye(128, dtype=np.float32)
    for q in range(_NROW):
        cst[:, 128 + q * _NROW + q] = 1.0
    return cst.astype(ml_dtypes.float16 if hasattr(ml_dtypes, "float16") else np.float16)


def _make_in_maps(net_output, target):
    net_output = np.ascontiguousarray(net_output, dtype=np.float32)
    target = np.ascontiguousarray(target, dtype=np.int32)
    cst = np.ascontiguousarray(_make_consts())
    in_maps = []
    for k in range(_NCORES):
        xs = net_output[_NB * k: _NB * (k + 1)].reshape(_NB, _C, _P, _FREE)
        ts = target[_NB * k: _NB * (k + 1), 0].reshape(_NB, _P, _FREE)
        in_maps.append({"x": np.ascontiguousarray(xs), "t": np.ascontiguousarray(ts),
                        "c": cst})
    return in_maps


def _combine(results, bare_weight):
    # results: list of dicts with 'o' [NROW, CHUNK] per core
    D = np.zeros((_B, _C), np.float64)
    Hc = np.zeros((_B, _C), np.float64)
    for k, r in enumerate(results):
        o = r["o"].astype(np.float64).sum(axis=1).reshape(_NB, _NQ)
        for bb in range(_NB):
            gb = _NB * k + bb
            D[gb, 1:4] = o[bb, 0:3]
            Hc[gb, 1:4] = o[bb, 3:6]

    bw = bare_weight.astype(np.float64)
    sig = 1.0 / (1.0 + np.exp(-bw))
    w = sig / sig.mean(axis=0, keepdims=True)
    w = w / (1.0 + np.e)  # fixed_w for classes >= 1
    loss = (w[:, 1:] * (-D[:, 1:] - Hc[:, 1:])).sum() / _HW
    return np.float32(loss)


def _run(net_output, target, bare_weight, **spmd_kwargs):
    from concourse.bass_utils import run_bass_kernel_spmd

    nc = _get_nc()
    in_maps = _make_in_maps(net_output, target)
    res = run_bass_kernel_spmd(nc, in_maps, core_ids=list(range(_NCORES)), **spmd_kwargs)
    return _combine(res.results, np.asarray(bare_weight)), res


def kernel(net_output, target, bare_weight):
    loss, _ = _run(np.asarray(net_output), np.asarray(target), np.asarray(bare_weight))
    return loss
